# revision 22
# baseline (speedup 1.0000x reference)
"""NLL loss (3x3 mahalanobis + logdet + mean) on 8 TRN2 cores.

Math per row (inputs l0..l5 -> L lower-tri, M = L@L.T + eps*I):
  Cholesky of M: M = G G^T, computed entrywise.
  quad  = |G^{-1} diff|^2         (== diff^T M^{-1} diff)
  logdet = 2*ln(g00*g11*g22)
Outputs: [mean(quad), mean(logdet), mean(frc_var)].

Data-parallel: rows padded 4,000,000 -> 4,000,768 = 8 cores * 128
partitions * 3907. Host lays out SoA [9, 128, 3907] per core
(rows 0-2 = tgt-prd, rows 3-8 = frc_var). Each core returns raw sums
[quad_sum, ln(g00*g11*g22)_sum, var_sum]; host combines and corrects
for the 768 zero-pad rows (each contributes 1.5*ln(eps) to the ln sum).
"""

import numpy as np

N = 4_000_000
P = 128
W = 3907                      # free width per partition per core
NCORES = 8
PER_CORE = P * W              # 500,096
NPAD = NCORES * PER_CORE      # 4,000,768
EPS = 1e-3
F32 = np.float32

_CACHE: dict = {}
RUN_KWARGS: dict = {}


def _build_program():
    from contextlib import ExitStack

    import concourse.bacc as bacc
    import concourse.tile as tile
    from concourse import mybir

    dt = mybir.dt
    Alu = mybir.AluOpType
    Act = mybir.ActivationFunctionType

    nc = bacc.Bacc(
        "TRN2",
        target_bir_lowering=False,
        debug=False,
        num_devices=NCORES,
    )

    din = nc.dram_tensor("din", [P, 9, W], dt.float32, kind="ExternalInput").ap()
    dout = nc.dram_tensor("dout", [1, 3], dt.float32, kind="ExternalOutput").ap()

    widths = [512] * 7 + [323]
    offs = [512 * i for i in range(8)]
    nch = len(widths)

    with tile.TileContext(nc) as tc, ExitStack() as ctx:
        pool_in = ctx.enter_context(tc.tile_pool(name="in", bufs=3))
        pool_var = ctx.enter_context(tc.tile_pool(name="var", bufs=2))
        pool_act = ctx.enter_context(tc.tile_pool(name="act", bufs=24))
        pool_dve = ctx.enter_context(tc.tile_pool(name="dve", bufs=34))
        pool_misc = ctx.enter_context(tc.tile_pool(name="misc", bufs=1))
        pool_psum = ctx.enter_context(tc.tile_pool(name="ps", bufs=1, space="PSUM"))

        # persistent accumulators / helpers
        acc = pool_misc.tile([P, nch, 5], dt.float32)    # per-chunk accum cols
        ones = pool_misc.tile([P, 1], dt.float32)
        eps_t = pool_misc.tile([P, 1], dt.float32)
        dead = pool_misc.tile([P, 6, 512], dt.float32)   # dead ACT outputs
        qln = pool_misc.tile([P, 3], dt.float32)
        fin = pool_misc.tile([1, 3], dt.float32)
        pfin = pool_psum.tile([1, 3], dt.float32)

        nc.gpsimd.memset(ones[:], 1.0)
        nc.gpsimd.memset(eps_t[:], EPS)

        for j, (off, F) in enumerate(zip(offs, widths)):
            dtile = pool_in.tile([P, 3, 512], dt.float32)
            nc.sync.dma_start(
                out=dtile[:, :, 0:F], in_=din[:, 0:3, off : off + F]
            )
            var6 = pool_var.tile([P, 6, 512], dt.float32)
            nc.sync.dma_start(
                out=var6[:, :, 0:F], in_=din[:, 3:9, off : off + F]
            )
            d0, d1, d2 = (dtile[:, k, 0:F] for k in range(3))
            va, vb, vc, vd, ve, vf = (var6[:, k, 0:F] for k in range(6))

            def sq(x, out_pool=pool_act, F=F):
                o = out_pool.tile([P, F], dt.float32)
                nc.scalar.activation(o[:], x[:], Act.Square)
                return o

            def tt(op, x, y, F=F):
                o = pool_dve.tile([P, F], dt.float32)
                nc.vector.tensor_tensor(o[:], x[:], y[:], op)
                return o

            def sqrt_eps(x, F=F):
                o = pool_act.tile([P, F], dt.float32)
                nc.scalar.activation(o[:], x[:], Act.Sqrt, bias=eps_t[:])
                return o

            def recip(x, F=F):
                o = pool_dve.tile([P, F], dt.float32)
                nc.vector.reciprocal(o[:], x[:])
                return o

            # var sum: one ACT Copy pass over the whole [P,6,F] block
            nc.scalar.activation(
                dead[:, 0:6, 0:F], var6[:, :, 0:F], Act.Copy,
                accum_out=acc[:, j, 4:5],
            )

            a2 = sq(va)
            b2 = sq(vb)
            c2 = sq(vc)
            dsq = sq(vd)
            e2 = sq(ve)
            fsq = sq(vf)

            g00 = sqrt_eps(a2)                     # sqrt(a^2 + eps)
            r00 = recip(g00)
            m11p = tt(Alu.add, fsq, b2)
            m22p = tt(Alu.add, e2, dsq)
            m22q = tt(Alu.add, m22p, c2)
            m01 = tt(Alu.mult, va, vf)
            m02 = tt(Alu.mult, va, ve)
            bd = tt(Alu.mult, vb, vd)
            ef = tt(Alu.mult, ve, vf)
            m12 = tt(Alu.add, ef, bd)
            g10 = tt(Alu.mult, m01, r00)
            g20 = tt(Alu.mult, m02, r00)
            g10s = sq(g10)
            s11 = tt(Alu.subtract, m11p, g10s)
            g11 = sqrt_eps(s11)
            r11 = recip(g11)
            gg = tt(Alu.mult, g10, g20)
            a21 = tt(Alu.subtract, m12, gg)
            g21 = tt(Alu.mult, a21, r11)
            g20s = sq(g20)
            g21s = sq(g21)
            s22a = tt(Alu.subtract, m22q, g20s)
            s22 = tt(Alu.subtract, s22a, g21s)
            g22 = sqrt_eps(s22)
            r22 = recip(g22)

            # logdet partial: ln(g00*g11*g22), accumulated along free dim
            r1 = tt(Alu.mult, g00, g11)
            r2 = tt(Alu.mult, r1, g22)
            nc.scalar.activation(
                dead[:, 0, 0:F], r2[:], Act.Ln,
                accum_out=acc[:, j, 3:4],
            )

            # forward solve G y = diff
            y0 = tt(Alu.mult, d0, r00)
            gy = tt(Alu.mult, g10, y0)
            d1m = tt(Alu.subtract, d1, gy)
            y1 = tt(Alu.mult, d1m, r11)
            g20y0 = tt(Alu.mult, g20, y0)
            d2m = tt(Alu.subtract, d2, g20y0)
            g21y1 = tt(Alu.mult, g21, y1)
            d2n = tt(Alu.subtract, d2m, g21y1)
            y2 = tt(Alu.mult, d2n, r22)

            # quad partials: sum(y_i^2) via ACT accum
            for yi, col in ((y0, 0), (y1, 1), (y2, 2)):
                nc.scalar.activation(
                    dead[:, 0, 0:F], yi[:], Act.Square,
                    accum_out=acc[:, j, col : col + 1],
                )

        # final reductions
        nc.vector.tensor_reduce(
            qln[:, 0:1], acc[:, :, 0:3], mybir.AxisListType.XY, Alu.add
        )
        nc.vector.tensor_reduce(
            qln[:, 1:2], acc[:, :, 3:4], mybir.AxisListType.XY, Alu.add
        )
        nc.vector.tensor_reduce(
            qln[:, 2:3], acc[:, :, 4:5], mybir.AxisListType.XY, Alu.add
        )
        nc.tensor.matmul(pfin[:], ones[:], qln[:], start=True, stop=True)
        nc.scalar.copy(fin[0:1, 0:3], pfin[0:1, 0:3])
        nc.sync.dma_start(out=dout[:], in_=fin[:])

    nc.finalize()
    return nc


def _get_nc():
    if "nc" not in _CACHE:
        _CACHE["nc"] = _build_program()
    return _CACHE["nc"]


def kernel(prd_frc: np.ndarray, tgt_frc: np.ndarray, frc_var: np.ndarray) -> np.ndarray:
    from concourse.bass_utils import run_bass_kernel_spmd

    nc = _get_nc()

    big = np.zeros((9, NPAD), dtype=F32)
    np.subtract(tgt_frc.T, prd_frc.T, out=big[0:3, :N])
    big[3:9, :N] = frc_var.T
    per_core = big.reshape(9, NCORES, P, W).transpose(1, 2, 0, 3)
    in_maps = [
        {"din": np.ascontiguousarray(per_core[c])} for c in range(NCORES)
    ]

    res = run_bass_kernel_spmd(nc, in_maps, list(range(NCORES)), **RUN_KWARGS)
    _CACHE["last_results"] = res

    sums = np.array([r["dout"][0] for r in res.results], dtype=np.float64)
    q_sum, ln_sum, v_sum = sums.sum(axis=0)
    npad_rows = NPAD - N
    logdet_sum = 2.0 * ln_sum - npad_rows * 3.0 * np.log(EPS)
    return np.array(
        [q_sum / N, logdet_sum / N, v_sum / (6 * N)], dtype=F32
    )


# revision 30
# speedup vs baseline: 1.2122x; 1.2122x over previous
"""NLL loss (3x3 mahalanobis + logdet + mean) on 8 TRN2 cores.

Math per row (inputs l0..l5 -> L lower-tri, M = L@L.T + eps*I):
  Cholesky of M: M = G G^T, computed entrywise.
  quad  = |G^{-1} diff|^2         (== diff^T M^{-1} diff)
  logdet = 2*ln(g00*g11*g22)
Outputs: [mean(quad), mean(logdet), mean(frc_var)].

Data-parallel: rows padded 4,000,000 -> 4,000,768 = 8 cores * 128
partitions * 3907. Host lays out SoA [9, 128, 3907] per core
(rows 0-2 = tgt-prd, rows 3-8 = frc_var). Each core returns raw sums
[quad_sum, ln(g00*g11*g22)_sum, var_sum]; host combines and corrects
for the 768 zero-pad rows (each contributes 1.5*ln(eps) to the ln sum).
"""

import numpy as np

N = 4_000_000
P = 128
W = 3907                      # free width per partition per core
NCORES = 8
PER_CORE = P * W              # 500,096
NPAD = NCORES * PER_CORE      # 4,000,768
EPS = 1e-3
F32 = np.float32

_CACHE: dict = {}
RUN_KWARGS: dict = {}


def _build_program():
    from contextlib import ExitStack

    import concourse.bacc as bacc
    import concourse.tile as tile
    from concourse import mybir

    dt = mybir.dt
    Alu = mybir.AluOpType
    Act = mybir.ActivationFunctionType

    nc = bacc.Bacc(
        "TRN2",
        target_bir_lowering=False,
        debug=False,
        num_devices=NCORES,
    )

    din = nc.dram_tensor("din", [P, 9, W], dt.float32, kind="ExternalInput").ap()
    dout = nc.dram_tensor("dout", [1, 3], dt.float32, kind="ExternalOutput").ap()

    widths = [512] * 7 + [323]
    offs = [512 * i for i in range(8)]
    nch = len(widths)

    with tile.TileContext(nc) as tc, ExitStack() as ctx:
        pool_in = ctx.enter_context(tc.tile_pool(name="in", bufs=3))
        pool_var = ctx.enter_context(tc.tile_pool(name="var", bufs=2))
        pool_act = ctx.enter_context(tc.tile_pool(name="act", bufs=24))
        pool_dve = ctx.enter_context(tc.tile_pool(name="dve", bufs=34))
        pool_misc = ctx.enter_context(tc.tile_pool(name="misc", bufs=1))
        pool_psum = ctx.enter_context(tc.tile_pool(name="ps", bufs=1, space="PSUM"))

        # persistent accumulators / helpers
        acc = pool_misc.tile([P, nch, 7], dt.float32)    # per-chunk accum cols
        ones = pool_misc.tile([P, 1], dt.float32)
        eps_t = pool_misc.tile([P, 1], dt.float32)
        dead = pool_misc.tile([P, 6, 512], dt.float32)   # dead ACT outputs
        qln = pool_misc.tile([P, 3], dt.float32)
        fin = pool_misc.tile([1, 3], dt.float32)
        pfin = pool_psum.tile([1, 3], dt.float32)

        nc.gpsimd.memset(ones[:], 1.0)
        nc.gpsimd.memset(eps_t[:], EPS)

        for j, (off, F) in enumerate(zip(offs, widths)):
            dtile = pool_in.tile([P, 3, 512], dt.float32)
            nc.sync.dma_start(
                out=dtile[:, :, 0:F], in_=din[:, 0:3, off : off + F]
            )
            var6 = pool_var.tile([P, 6, 512], dt.float32)
            nc.sync.dma_start(
                out=var6[:, :, 0:F], in_=din[:, 3:9, off : off + F]
            )
            d0, d1, d2 = (dtile[:, k, 0:F] for k in range(3))
            va, vb, vc, vd, ve, vf = (var6[:, k, 0:F] for k in range(6))

            def sq(x, out_pool=pool_act, F=F):
                o = out_pool.tile([P, F], dt.float32)
                nc.scalar.activation(o[:], x[:], Act.Square)
                return o

            def tt(op, x, y, F=F):
                o = pool_dve.tile([P, F], dt.float32)
                nc.vector.tensor_tensor(o[:], x[:], y[:], op)
                return o

            def ln_eps(x, col, F=F):
                # o = ln(x + eps); accum_out sums ln over free dim (logdet partial)
                o = pool_act.tile([P, F], dt.float32)
                nc.scalar.activation(
                    o[:], x[:], Act.Ln, bias=eps_t[:],
                    accum_out=acc[:, j, col : col + 1],
                )
                return o

            def rsqrt_ln(lnx, F=F):
                # exp(-0.5*ln(x+eps)) == 1/sqrt(x+eps)
                o = pool_act.tile([P, F], dt.float32)
                nc.scalar.activation(o[:], lnx[:], Act.Exp, scale=-0.5)
                return o

            # var sum: one ACT Copy pass over the whole [P,6,F] block
            nc.scalar.activation(
                dead[:, 0:6, 0:F], var6[:, :, 0:F], Act.Copy,
                accum_out=acc[:, j, 6:7],
            )

            a2 = sq(va)
            b2 = sq(vb)
            c2 = sq(vc)
            dsq = sq(vd)
            e2 = sq(ve)
            fsq = sq(vf)

            ln00 = ln_eps(a2, 3)
            r00 = rsqrt_ln(ln00)                   # 1/sqrt(a^2 + eps)
            m11p = tt(Alu.add, fsq, b2)
            m22p = tt(Alu.add, e2, dsq)
            m22q = tt(Alu.add, m22p, c2)
            m01 = tt(Alu.mult, va, vf)
            m02 = tt(Alu.mult, va, ve)
            bd = tt(Alu.mult, vb, vd)
            ef = tt(Alu.mult, ve, vf)
            m12 = tt(Alu.add, ef, bd)
            g10 = tt(Alu.mult, m01, r00)
            g20 = tt(Alu.mult, m02, r00)
            g10s = sq(g10)
            s11 = tt(Alu.subtract, m11p, g10s)
            ln11 = ln_eps(s11, 4)
            r11 = rsqrt_ln(ln11)
            gg = tt(Alu.mult, g10, g20)
            a21 = tt(Alu.subtract, m12, gg)
            g21 = tt(Alu.mult, a21, r11)
            g20s = sq(g20)
            g21s = sq(g21)
            s22a = tt(Alu.subtract, m22q, g20s)
            s22 = tt(Alu.subtract, s22a, g21s)
            ln22 = ln_eps(s22, 5)
            r22 = rsqrt_ln(ln22)

            # forward solve G y = diff
            y0 = tt(Alu.mult, d0, r00)
            gy = tt(Alu.mult, g10, y0)
            d1m = tt(Alu.subtract, d1, gy)
            y1 = tt(Alu.mult, d1m, r11)
            g20y0 = tt(Alu.mult, g20, y0)
            d2m = tt(Alu.subtract, d2, g20y0)
            g21y1 = tt(Alu.mult, g21, y1)
            d2n = tt(Alu.subtract, d2m, g21y1)
            y2 = tt(Alu.mult, d2n, r22)

            # quad partials: sum(y_i^2) via ACT accum
            for yi, col in ((y0, 0), (y1, 1), (y2, 2)):
                nc.scalar.activation(
                    dead[:, 0, 0:F], yi[:], Act.Square,
                    accum_out=acc[:, j, col : col + 1],
                )

        # final reductions
        nc.vector.tensor_reduce(
            qln[:, 0:1], acc[:, :, 0:3], mybir.AxisListType.XY, Alu.add
        )
        nc.vector.tensor_reduce(
            qln[:, 1:2], acc[:, :, 3:6], mybir.AxisListType.XY, Alu.add
        )
        nc.vector.tensor_reduce(
            qln[:, 2:3], acc[:, :, 6:7], mybir.AxisListType.XY, Alu.add
        )
        nc.tensor.matmul(pfin[:], ones[:], qln[:], start=True, stop=True)
        nc.scalar.copy(fin[0:1, 0:3], pfin[0:1, 0:3])
        nc.sync.dma_start(out=dout[:], in_=fin[:])

    nc.finalize()
    return nc


def _get_nc():
    if "nc" not in _CACHE:
        _CACHE["nc"] = _build_program()
    return _CACHE["nc"]


def kernel(prd_frc: np.ndarray, tgt_frc: np.ndarray, frc_var: np.ndarray) -> np.ndarray:
    from concourse.bass_utils import run_bass_kernel_spmd

    nc = _get_nc()

    big = np.zeros((9, NPAD), dtype=F32)
    np.subtract(tgt_frc.T, prd_frc.T, out=big[0:3, :N])
    big[3:9, :N] = frc_var.T
    per_core = big.reshape(9, NCORES, P, W).transpose(1, 2, 0, 3)
    in_maps = [
        {"din": np.ascontiguousarray(per_core[c])} for c in range(NCORES)
    ]

    res = run_bass_kernel_spmd(nc, in_maps, list(range(NCORES)), **RUN_KWARGS)
    _CACHE["last_results"] = res

    sums = np.array([r["dout"][0] for r in res.results], dtype=np.float64)
    q_sum, ln_sum, v_sum = sums.sum(axis=0)
    npad_rows = NPAD - N
    logdet_sum = ln_sum - npad_rows * 3.0 * np.log(EPS)
    return np.array(
        [q_sum / N, logdet_sum / N, v_sum / (6 * N)], dtype=F32
    )


# revision 31
# speedup vs baseline: 1.5080x; 1.2440x over previous
"""NLL loss (3x3 mahalanobis + logdet + mean) on 8 TRN2 cores.

Math per row (inputs l0..l5 -> L lower-tri, M = L@L.T + eps*I):
  Cholesky of M: M = G G^T, computed entrywise.
  quad  = |G^{-1} diff|^2         (== diff^T M^{-1} diff)
  logdet = 2*ln(g00*g11*g22)
Outputs: [mean(quad), mean(logdet), mean(frc_var)].

Data-parallel: rows padded 4,000,000 -> 4,000,768 = 8 cores * 128
partitions * 3907. Host lays out SoA [9, 128, 3907] per core
(rows 0-2 = tgt-prd, rows 3-8 = frc_var). Each core returns raw sums
[quad_sum, ln(g00*g11*g22)_sum, var_sum]; host combines and corrects
for the 768 zero-pad rows (each contributes 1.5*ln(eps) to the ln sum).
"""

import numpy as np

N = 4_000_000
P = 128
W = 3907                      # free width per partition per core
NCORES = 8
PER_CORE = P * W              # 500,096
NPAD = NCORES * PER_CORE      # 4,000,768
EPS = 1e-3
F32 = np.float32

_CACHE: dict = {}
RUN_KWARGS: dict = {}


def _patch_act_tables():
    """Force the act-table-load pass to place Ln/Exp/Square/Copy in the single
    set that holds all four (natural_log_exp_and_others), so the ACT engine
    loads its table once instead of thrashing between ln- and exp-sets."""
    import concourse.bacc as bacc_mod
    from concourse import mybir

    if getattr(bacc_mod, "_act_tables_patched", False):
        return
    orig = bacc_mod.get_activation_tables
    AF = mybir.ActivationFunctionType
    wanted = {AF.Ln, AF.Exp, AF.Square, AF.Copy}

    def patched(arch, _orig=orig):
        out = {}
        for name, funcs in _orig(arch).items():
            if name == "natural_log_exp_and_others":
                out[name] = set(funcs)
            else:
                out[name] = set(funcs) - wanted
        return out

    bacc_mod.get_activation_tables = patched
    bacc_mod._act_tables_patched = True


def _build_program():
    from contextlib import ExitStack

    import concourse.bacc as bacc
    import concourse.tile as tile
    from concourse import mybir

    _patch_act_tables()

    dt = mybir.dt
    Alu = mybir.AluOpType
    Act = mybir.ActivationFunctionType

    nc = bacc.Bacc(
        "TRN2",
        target_bir_lowering=False,
        debug=False,
        num_devices=NCORES,
    )

    din = nc.dram_tensor("din", [P, 9, W], dt.float32, kind="ExternalInput").ap()
    dout = nc.dram_tensor("dout", [1, 3], dt.float32, kind="ExternalOutput").ap()

    widths = [512] * 7 + [323]
    offs = [512 * i for i in range(8)]
    nch = len(widths)

    with tile.TileContext(nc) as tc, ExitStack() as ctx:
        pool_in = ctx.enter_context(tc.tile_pool(name="in", bufs=3))
        pool_var = ctx.enter_context(tc.tile_pool(name="var", bufs=2))
        pool_act = ctx.enter_context(tc.tile_pool(name="act", bufs=24))
        pool_dve = ctx.enter_context(tc.tile_pool(name="dve", bufs=34))
        pool_misc = ctx.enter_context(tc.tile_pool(name="misc", bufs=1))
        pool_psum = ctx.enter_context(tc.tile_pool(name="ps", bufs=1, space="PSUM"))

        # persistent accumulators / helpers
        acc = pool_misc.tile([P, nch, 7], dt.float32)    # per-chunk accum cols
        ones = pool_misc.tile([P, 1], dt.float32)
        eps_t = pool_misc.tile([P, 1], dt.float32)
        dead = pool_misc.tile([P, 6, 512], dt.float32)   # dead ACT outputs
        qln = pool_misc.tile([P, 3], dt.float32)
        fin = pool_misc.tile([1, 3], dt.float32)
        pfin = pool_psum.tile([1, 3], dt.float32)

        nc.gpsimd.memset(ones[:], 1.0)
        nc.gpsimd.memset(eps_t[:], EPS)

        for j, (off, F) in enumerate(zip(offs, widths)):
            dtile = pool_in.tile([P, 3, 512], dt.float32)
            nc.sync.dma_start(
                out=dtile[:, :, 0:F], in_=din[:, 0:3, off : off + F]
            )
            var6 = pool_var.tile([P, 6, 512], dt.float32)
            nc.sync.dma_start(
                out=var6[:, :, 0:F], in_=din[:, 3:9, off : off + F]
            )
            d0, d1, d2 = (dtile[:, k, 0:F] for k in range(3))
            va, vb, vc, vd, ve, vf = (var6[:, k, 0:F] for k in range(6))

            def sq(x, out_pool=pool_act, F=F):
                o = out_pool.tile([P, F], dt.float32)
                nc.scalar.activation(o[:], x[:], Act.Square)
                return o

            def tt(op, x, y, F=F):
                o = pool_dve.tile([P, F], dt.float32)
                nc.vector.tensor_tensor(o[:], x[:], y[:], op)
                return o

            def ln_eps(x, col, F=F):
                # o = ln(x + eps); accum_out sums ln over free dim (logdet partial)
                o = pool_act.tile([P, F], dt.float32)
                nc.scalar.activation(
                    o[:], x[:], Act.Ln, bias=eps_t[:],
                    accum_out=acc[:, j, col : col + 1],
                )
                return o

            def rsqrt_ln(lnx, F=F):
                # exp(-0.5*ln(x+eps)) == 1/sqrt(x+eps)
                o = pool_act.tile([P, F], dt.float32)
                nc.scalar.activation(o[:], lnx[:], Act.Exp, scale=-0.5)
                return o

            # var sum: one ACT Copy pass over the whole [P,6,F] block
            nc.scalar.activation(
                dead[:, 0:6, 0:F], var6[:, :, 0:F], Act.Copy,
                accum_out=acc[:, j, 6:7],
            )

            a2 = sq(va)
            b2 = sq(vb)
            c2 = sq(vc)
            dsq = sq(vd)
            e2 = sq(ve)
            fsq = sq(vf)

            ln00 = ln_eps(a2, 3)
            r00 = rsqrt_ln(ln00)                   # 1/sqrt(a^2 + eps)
            m11p = tt(Alu.add, fsq, b2)
            m22p = tt(Alu.add, e2, dsq)
            m22q = tt(Alu.add, m22p, c2)
            m01 = tt(Alu.mult, va, vf)
            m02 = tt(Alu.mult, va, ve)
            bd = tt(Alu.mult, vb, vd)
            ef = tt(Alu.mult, ve, vf)
            m12 = tt(Alu.add, ef, bd)
            g10 = tt(Alu.mult, m01, r00)
            g20 = tt(Alu.mult, m02, r00)
            g10s = sq(g10)
            s11 = tt(Alu.subtract, m11p, g10s)
            ln11 = ln_eps(s11, 4)
            r11 = rsqrt_ln(ln11)
            gg = tt(Alu.mult, g10, g20)
            a21 = tt(Alu.subtract, m12, gg)
            g21 = tt(Alu.mult, a21, r11)
            g20s = sq(g20)
            g21s = sq(g21)
            s22a = tt(Alu.subtract, m22q, g20s)
            s22 = tt(Alu.subtract, s22a, g21s)
            ln22 = ln_eps(s22, 5)
            r22 = rsqrt_ln(ln22)

            # forward solve G y = diff
            y0 = tt(Alu.mult, d0, r00)
            gy = tt(Alu.mult, g10, y0)
            d1m = tt(Alu.subtract, d1, gy)
            y1 = tt(Alu.mult, d1m, r11)
            g20y0 = tt(Alu.mult, g20, y0)
            d2m = tt(Alu.subtract, d2, g20y0)
            g21y1 = tt(Alu.mult, g21, y1)
            d2n = tt(Alu.subtract, d2m, g21y1)
            y2 = tt(Alu.mult, d2n, r22)

            # quad partials: sum(y_i^2) via ACT accum
            for yi, col in ((y0, 0), (y1, 1), (y2, 2)):
                nc.scalar.activation(
                    dead[:, 0, 0:F], yi[:], Act.Square,
                    accum_out=acc[:, j, col : col + 1],
                )

        # final reductions
        nc.vector.tensor_reduce(
            qln[:, 0:1], acc[:, :, 0:3], mybir.AxisListType.XY, Alu.add
        )
        nc.vector.tensor_reduce(
            qln[:, 1:2], acc[:, :, 3:6], mybir.AxisListType.XY, Alu.add
        )
        nc.vector.tensor_reduce(
            qln[:, 2:3], acc[:, :, 6:7], mybir.AxisListType.XY, Alu.add
        )
        nc.tensor.matmul(pfin[:], ones[:], qln[:], start=True, stop=True)
        nc.scalar.copy(fin[0:1, 0:3], pfin[0:1, 0:3])
        nc.sync.dma_start(out=dout[:], in_=fin[:])

    nc.finalize()
    return nc


def _get_nc():
    if "nc" not in _CACHE:
        _CACHE["nc"] = _build_program()
    return _CACHE["nc"]


def kernel(prd_frc: np.ndarray, tgt_frc: np.ndarray, frc_var: np.ndarray) -> np.ndarray:
    from concourse.bass_utils import run_bass_kernel_spmd

    nc = _get_nc()

    big = np.zeros((9, NPAD), dtype=F32)
    np.subtract(tgt_frc.T, prd_frc.T, out=big[0:3, :N])
    big[3:9, :N] = frc_var.T
    per_core = big.reshape(9, NCORES, P, W).transpose(1, 2, 0, 3)
    in_maps = [
        {"din": np.ascontiguousarray(per_core[c])} for c in range(NCORES)
    ]

    res = run_bass_kernel_spmd(nc, in_maps, list(range(NCORES)), **RUN_KWARGS)
    _CACHE["last_results"] = res

    sums = np.array([r["dout"][0] for r in res.results], dtype=np.float64)
    q_sum, ln_sum, v_sum = sums.sum(axis=0)
    npad_rows = NPAD - N
    logdet_sum = ln_sum - npad_rows * 3.0 * np.log(EPS)
    return np.array(
        [q_sum / N, logdet_sum / N, v_sum / (6 * N)], dtype=F32
    )
